# revision 1
# baseline (speedup 1.0000x reference)
"""Complex 3D+temporal conv (ComplexPadConv3Dt) on 8 Trainium2 NeuronCores.

Strategy (hardcoded for B=2, T=8, Z=20, Y=64, X=64, C=2, F1=F=32, k=3):
 - Pure data-parallel sharding: 8 cores = B(2) x X-quarters(4). Each core
   computes its (b, 16-wide x slab) including halo; no collectives.
 - Host: weight projection, symmetric padding, channel-separated relayout
   with a (dz:2, dy:3)-shifted 6x partition stack, final gather.
 - All matmuls are float32r (full-rate PE, ~2e-4 accuracy vs fp32).
   fp32r tolerates only row tile_positions (col positions miscompile), so
   concurrency comes from 4 row-group tiles writing 4 separate PSUM banks.
 - Spatial conv: contraction K=24 = (dz:2, dy:3, c:2, re/im:2) plus a K=12
   wave reading the dz=0 block at a z+2 free offset; 6 accumulating waves
   cover all 27 taps; y-half chunks are sequential.
 - Temporal conv: K=64 channel contraction, 3 taps accumulated, row tiles
   (0,0)/(64,0) for the two y-half chunks.
 - Output: DVE 32x32-block transposes per PSUM bank yield per-partition
   (x,f)-contiguous 2KB runs for efficient HBM writes; scalar engine takes
   the partition-preserving PSUM evacuations, DVE the cross-quadrant ones.
"""

import numpy as np

import concourse.bass as bass
import concourse.bacc as bacc
import concourse.mybir as mybir
from concourse import tile
from concourse.bass_utils import run_bass_kernel_spmd

# Problem constants
B, T, Z, Y, X, C = 2, 8, 20, 64, 64, 2
F1, F = 32, 32
KZ = KY = KX = 3
KT = 3

# Sharding / tiling
XC = 16          # output x columns per core
NXC = X // XC    # 4 x-chunks
XI = XC + 2      # input x columns per core (halo)
ZB = 4           # z rows per block
NZB = Z // ZB    # 5 blocks
ZI = ZB + 2      # slab z extent per block (dz reads need +2)
YP, ZP = Y + 2, Z + 2

F32 = mybir.dt.float32
F32R = mybir.dt.float32r

_NC_CACHE = {}


def _project(wr, wi, zero_mean):
    wr = wr.astype(np.float64)
    wi = wi.astype(np.float64)
    ax = (0, 1, 2, 3)
    if zero_mean:
        wr = wr - wr.mean(ax, keepdims=True)
        wi = wi - wi.mean(ax, keepdims=True)
    norm = np.sqrt((wr * wr + wi * wi).sum(ax, keepdims=True))
    s = 1.0 / np.maximum(norm, 1.0)
    return wr * s, wi * s


def _spatial_lhsT(wsr, wsi):
    """[128, 6*64] f32. col block w = dx*2 + grp.

    grp=0: rows 32g + dz*12 + dy*4 + c*2 + part (dz in {0,1}), K=24
    grp=1: rows 32g + dy*4 + c*2 + part (the dz=2 tap), K=12
    """
    w = np.zeros((32, 6 * 64), np.float64)
    for dx in range(KX):
        for grp in range(2):
            wcol = (dx * 2 + grp) * 64
            dzs = (0, 1) if grp == 0 else (2,)
            for dzi, dz in enumerate(dzs):
                for dy in range(KY):
                    for c in range(C):
                        rr = dzi * 12 + dy * 4 + c * 2 + 0
                        ri = dzi * 12 + dy * 4 + c * 2 + 1
                        w[rr, wcol + 0:wcol + 32] = wsr[dz, dy, dx, c, :]
                        w[rr, wcol + 32:wcol + 64] = wsi[dz, dy, dx, c, :]
                        w[ri, wcol + 0:wcol + 32] = -wsi[dz, dy, dx, c, :]
                        w[ri, wcol + 32:wcol + 64] = wsr[dz, dy, dx, c, :]
    out = np.zeros((128, 6 * 64), np.float32)
    for g in range(4):
        out[32 * g:32 * g + 32] = w
    return out


def _temporal_lhsT(wtr, wti):
    """[128, 5*64] f32. rows 64d + q*32 + f1 (q=0 spr, 1 spi); cols v*64 + part'*32 + f.

    variants v: [wt0, wt1, wt2, wt0+wt1, wt1+wt2]
    """
    wtr = wtr.reshape(KT, F1, F)
    wti = wti.reshape(KT, F1, F)
    variants = [
        (wtr[0], wti[0]),
        (wtr[1], wti[1]),
        (wtr[2], wti[2]),
        (wtr[0] + wtr[1], wti[0] + wti[1]),
        (wtr[1] + wtr[2], wti[1] + wti[2]),
    ]
    w = np.zeros((64, 5 * 64), np.float64)
    for v, (vr, vi) in enumerate(variants):
        w[0:32, v * 64 + 0:v * 64 + 32] = vr          # spr -> yr
        w[0:32, v * 64 + 32:v * 64 + 64] = vi         # spr -> yi
        w[32:64, v * 64 + 0:v * 64 + 32] = -vi        # spi -> yr
        w[32:64, v * 64 + 32:v * 64 + 64] = vr        # spi -> yi
    out = np.zeros((128, 5 * 64), np.float32)
    out[0:64] = w
    out[64:128] = w
    return out


def _temporal_taps(t):
    if t == 0:
        return [(0, 3), (1, 2)]
    if t == T - 1:
        return [(T - 2, 0), (T - 1, 4)]
    return [(t - 1, 0), (t, 1), (t + 1, 2)]


def build_program():
    nc = bacc.Bacc(None, target_bir_lowering=False)

    xin = nc.declare_dram_parameter("xin", [24, T, XI, ZP, Y], F32R, isOutput=False)
    wsp = nc.declare_dram_parameter("wsp", [128, 6 * 64], F32R, isOutput=False)
    wtp = nc.declare_dram_parameter("wtp", [128, 5 * 64], F32R, isOutput=False)
    out_r = nc.declare_dram_parameter("out_r", [T, Z, Y, XC, F], F32, isOutput=True)
    out_i = nc.declare_dram_parameter("out_i", [T, Z, Y, XC, F], F32, isOutput=True)

    with tile.TileContext(nc) as tc:
        with (
            tc.tile_pool(name="wpool", bufs=1) as wpool,
            tc.tile_pool(name="slabs", bufs=2) as slab_pool,
            tc.tile_pool(name="slices", bufs=9) as slice_pool,
            tc.tile_pool(name="stage", bufs=3) as stage_pool,
            tc.tile_pool(name="psum", bufs=8, space="PSUM") as psum_pool,
        ):
            wsp_sb = wpool.tile([128, 6 * 64], F32R, name="wsp_sb", tag="wsp")
            wtp_sb = wpool.tile([128, 5 * 64], F32R, name="wtp_sb", tag="wtp")
            nc.sync.dma_start(out=wsp_sb[:], in_=wsp[:])
            nc.sync.dma_start(out=wtp_sb[:], in_=wtp[:])

            for zb in range(NZB):
                z0 = zb * ZB
                slices = []
                # ---- spatial phase: 2 quads of 4 t-slices ----
                for quad in range(2):
                    slab = slab_pool.tile([128, XI * ZI * Y], F32R, name="slab", tag="slab")
                    slab_v = slab.rearrange(
                        "p (x z y) -> p x z y", x=XI, z=ZI, y=Y
                    )
                    for g in range(4):
                        t = quad * 4 + g
                        nc.sync.dma_start(
                            out=slab_v[32 * g:32 * g + 24],
                            in_=xin[:, t, :, z0:z0 + ZI, :],
                        )
                    for z in range(ZB):
                        for g in range(4):
                            t = quad * 4 + g
                            if len(slices) <= t:
                                slices.append(
                                    slice_pool.tile([128, ZB * 512], F32R, name="sl", tag="sl")
                                )
                        for j in range(2):
                            banks = []
                            for g in range(4):
                                banks.append(psum_pool.tile([64, 512], F32, name="ps", tag="ps"))
                            for w in range(6):
                                dx, grp = w // 2, w % 2
                                kk = 24 if grp == 0 else 12
                                zoff = z if grp == 0 else z + 2
                                for g in range(4):
                                    nc.tensor.matmul(
                                        out=banks[g][:, :],
                                        lhsT=wsp_sb[
                                            32 * g:32 * g + kk,
                                            w * 64:(w + 1) * 64,
                                        ],
                                        rhs=slab_v[
                                            32 * g:32 * g + kk,
                                            dx:dx + XC,
                                            zoff,
                                            32 * j:32 * j + 32,
                                        ],
                                        start=(w == 0),
                                        stop=(w == 5),
                                        tile_position=(32 * g, 0),
                                    )
                            for g in range(4):
                                t = quad * 4 + g
                                dst = slices[t][64 * j:64 * j + 64, z * 512:(z + 1) * 512]
                                if j == 0:
                                    nc.scalar.copy(dst, banks[g][:])
                                else:
                                    nc.vector.tensor_copy(dst, banks[g][:])

                # ---- temporal phase ----
                for t in range(T):
                    stg = stage_pool.tile([128, ZB * 512], F32, name="stg", tag="stg")
                    taps = _temporal_taps(t)
                    for z in range(ZB):
                        bo = []
                        for d in range(2):
                            bo.append(psum_pool.tile([64, 512], F32, name="ps", tag="ps"))
                        for a, (s, v) in enumerate(taps):
                            first = a == 0
                            last = a == len(taps) - 1
                            for d in range(2):
                                nc.tensor.matmul(
                                    out=bo[d][:, :],
                                    lhsT=wtp_sb[
                                        64 * d:64 * d + 64,
                                        v * 64:(v + 1) * 64,
                                    ],
                                    rhs=slices[s][
                                        64 * d:64 * d + 64,
                                        z * 512:(z + 1) * 512,
                                    ],
                                    start=first,
                                    stop=last,
                                    tile_position=(64 * d, 0),
                                )
                        for d in range(2):
                            nc.vector.transpose(
                                stg[64 * d:64 * d + 64, z * 512:(z + 1) * 512],
                                bo[d][:],
                            )
                    # stage layout: partition 32a + r, free z*512 + x*32 + f
                    # a=0: yr y=r; a=1: yi y=r; a=2: yr y=32+r; a=3: yi y=32+r
                    for ab in range(4):
                        dst_t = out_r if ab % 2 == 0 else out_i
                        u = ab // 2
                        dst = dst_t[t, z0:z0 + ZB, 32 * u:32 * u + 32].rearrange(
                            "z r x f -> r z x f"
                        )
                        src = stg[32 * ab:32 * ab + 32, :].rearrange(
                            "p (z x f) -> p z x f", z=ZB, x=XC, f=F
                        )
                        nc.sync.dma_start(out=dst, in_=src)

    nc.finalize()
    return nc


def _prep_inputs(xr, xi, wxyz_r, wxyz_i, wt_r, wt_i):
    xr = np.asarray(xr, np.float32)
    xi = np.asarray(xi, np.float32)

    wsr, wsi = _project(np.asarray(wxyz_r, np.float64), np.asarray(wxyz_i, np.float64), True)
    wtr, wti = _project(np.asarray(wt_r, np.float64), np.asarray(wt_i, np.float64), False)
    wsp = _spatial_lhsT(wsr, wsi)
    wtp = _temporal_lhsT(wtr, wti)

    pads = [(0, 0), (0, 0), (1, 1), (1, 1), (1, 1), (0, 0)]
    xp = np.stack([np.pad(xr, pads, mode="symmetric"),
                   np.pad(xi, pads, mode="symmetric")])  # [2, B, T, ZP, YP, XP]
    in_maps = []
    for core in range(8):
        b, cx = divmod(core, NXC)
        xs = xp[:, b, :, :, :, XC * cx:XC * cx + XI, :]   # [2, T, ZP, YP, XI, C]
        blocks = []
        for dz in (0, 1):
            zi = np.minimum(np.arange(ZP) + dz, ZP - 1)
            zs = xs[:, :, zi]
            ys = np.stack([zs[:, :, :, dy:dy + Y] for dy in range(KY)], axis=1)
            blocks.append(ys)                       # [2, 3, T, ZP, Y, XI, C]
        bl = np.stack(blocks, axis=1)               # [part, dz, dy, T, ZP, Y, XI, C]
        bl = bl.transpose(1, 2, 7, 0, 3, 6, 4, 5)   # [dz, dy, c, part, T, XI, ZP, Y]
        xin = np.ascontiguousarray(bl.reshape(24, T, XI, ZP, Y), np.float32)
        in_maps.append({"xin": xin, "wsp": wsp, "wtp": wtp})
    return in_maps


def kernel(xr, xi, wxyz_r, wxyz_i, wt_r, wt_i):
    if "nc" not in _NC_CACHE:
        _NC_CACHE["nc"] = build_program()
    nc = _NC_CACHE["nc"]

    in_maps = _prep_inputs(xr, xi, wxyz_r, wxyz_i, wt_r, wt_i)
    res = run_bass_kernel_spmd(nc, in_maps, list(range(8)))

    yr = np.empty((B, T, Z, Y, X, F), np.float32)
    yi = np.empty((B, T, Z, Y, X, F), np.float32)
    for core in range(8):
        b, cx = divmod(core, NXC)
        yr[b, :, :, :, XC * cx:XC * cx + XC, :] = res.results[core]["out_r"]
        yi[b, :, :, :, XC * cx:XC * cx + XC, :] = res.results[core]["out_i"]
    return yr, yi



# revision 4
# speedup vs baseline: 1.2323x; 1.2323x over previous
"""Complex 3D+temporal conv (ComplexPadConv3Dt) on 8 Trainium2 NeuronCores.

Strategy (hardcoded for B=2, T=8, Z=20, Y=64, X=64, C=2, F1=F=32, k=3):
 - Pure data-parallel sharding: 8 cores = B(2) x X-quarters(4). Each core
   computes its (b, 16-wide x slab) including halo; no collectives.
 - All matmuls bf16 (rel err ~5e-3 vs the 2e-2 gate), PSUM accumulates f32.
 - The PE array is output-drain-bound (~128 PSUM elems/cycle shared across
   concurrent tiles), so the design minimizes matmuls per output tile:
   * Spatial conv: 2 accumulating matmuls per [64,512] output tile:
     K=72 covering (dz,dy) x (c,ri) taps for dx in {0,1} (dx=1 rows are
     x-preshifted copies), then K=36 covering dx=2 via a free-dim x offset
     on the same 36 base rows. dz/dy shifts are baked into the DRAM relayout.
   * The two y-halves (j) of each (t,z) run as concurrent column tiles
     (cols 0-63 / 64-127) accumulating into one [128,512] PSUM bank.
   * Temporal conv: K=64 contraction (q,f1), 3 taps accumulated; per tap a
     4-matmul wave covers (2 z) x (2 j) on disjoint PE quadrants.
 - Spatial PSUM evacuated by ScalarE as bf16 slices; temporal PSUM
   evacuated via DVE 32x32 block transposes giving per-partition
   (x,f)-contiguous 1KB HBM runs; outputs stored bf16, upcast on host.
"""

import numpy as np
import ml_dtypes

import concourse.bass as bass
import concourse.bacc as bacc
import concourse.mybir as mybir
from concourse import tile
from concourse.bass_utils import run_bass_kernel_spmd

# Problem constants
B, T, Z, Y, X, C = 2, 8, 20, 64, 64, 2
F1, F = 32, 32
KZ = KY = KX = 3
KT = 3

# Sharding / tiling
XC = 16          # output x columns per core
NXC = X // XC    # 4 x-chunks
XI = XC + 2      # input x columns per core (halo)
ZB = 4           # z rows per block
NZB = Z // ZB    # 5 blocks
NR = 72          # spatial contraction rows: 36 shiftable + 36 dx=1-preshifted

F32 = mybir.dt.float32
BF16 = mybir.dt.bfloat16
BF16NP = ml_dtypes.bfloat16

_NC_CACHE = {}


def _project(wr, wi, zero_mean):
    wr = wr.astype(np.float64)
    wi = wi.astype(np.float64)
    ax = (0, 1, 2, 3)
    if zero_mean:
        wr = wr - wr.mean(ax, keepdims=True)
        wi = wi - wi.mean(ax, keepdims=True)
    norm = np.sqrt((wr * wr + wi * wi).sum(ax, keepdims=True))
    s = 1.0 / np.maximum(norm, 1.0)
    return wr * s, wi * s


def _spatial_lhsT(wsr, wsi):
    """[128, 2*64] f32->bf16.

    Block 0 (cols 0-63, K=72): rows r = (dz*3+dy)*4 + c*2 + ri for dx=0
    (r<36) and the same +36 for dx=1. Block 1 (cols 64-127, K=36):
    rows 0-35 for dx=2. Cols: q'*32 + f (q'=0 -> yr, q'=1 -> yi).
    """
    w = np.zeros((128, 2 * 64), np.float64)
    for blk, dxs in ((0, (0, 1)), (1, (2,))):
        for dxi, dx in enumerate(dxs):
            for dz in range(KZ):
                for dy in range(KY):
                    for c in range(C):
                        r0 = dxi * 36 + (dz * 3 + dy) * 4 + c * 2
                        col = blk * 64
                        wr = wsr[dz, dy, dx, c, :]
                        wi = wsi[dz, dy, dx, c, :]
                        w[r0 + 0, col + 0:col + 32] = wr
                        w[r0 + 0, col + 32:col + 64] = wi
                        w[r0 + 1, col + 0:col + 32] = -wi
                        w[r0 + 1, col + 32:col + 64] = wr
    return w.astype(BF16NP)


def _temporal_lhsT(wtr, wti):
    """[128, 5*64] bf16. rows 64d + q*32 + f1 (q=0 spr, 1 spi); cols q'*32 + f.

    variants v: [wt0, wt1, wt2, wt0+wt1, wt1+wt2]
    """
    wtr = wtr.reshape(KT, F1, F)
    wti = wti.reshape(KT, F1, F)
    variants = [
        (wtr[0], wti[0]),
        (wtr[1], wti[1]),
        (wtr[2], wti[2]),
        (wtr[0] + wtr[1], wti[0] + wti[1]),
        (wtr[1] + wtr[2], wti[1] + wti[2]),
    ]
    w = np.zeros((64, 5 * 64), np.float64)
    for v, (vr, vi) in enumerate(variants):
        w[0:32, v * 64 + 0:v * 64 + 32] = vr          # spr -> yr
        w[0:32, v * 64 + 32:v * 64 + 64] = vi         # spr -> yi
        w[32:64, v * 64 + 0:v * 64 + 32] = -vi        # spi -> yr
        w[32:64, v * 64 + 32:v * 64 + 64] = vr        # spi -> yi
    out = np.zeros((128, 5 * 64), np.float64)
    out[0:64] = w
    out[64:128] = w
    return out.astype(BF16NP)


def _temporal_taps(t):
    if t == 0:
        return [(0, 3), (1, 2)]
    if t == T - 1:
        return [(T - 2, 0), (T - 1, 4)]
    return [(t - 1, 0), (t, 1), (t + 1, 2)]


def build_program():
    nc = bacc.Bacc(None, target_bir_lowering=False)

    xin = nc.declare_dram_parameter("xin", [NR, T, Z, XI, Y], BF16, isOutput=False)
    wsp = nc.declare_dram_parameter("wsp", [128, 2 * 64], BF16, isOutput=False)
    wtp = nc.declare_dram_parameter("wtp", [128, 5 * 64], BF16, isOutput=False)
    out_r = nc.declare_dram_parameter("out_r", [T, Z, Y, XC, F], BF16, isOutput=True)
    out_i = nc.declare_dram_parameter("out_i", [T, Z, Y, XC, F], BF16, isOutput=True)

    with tile.TileContext(nc) as tc:
        with (
            tc.tile_pool(name="wpool", bufs=1) as wpool,
            tc.tile_pool(name="slabs", bufs=12) as slab_pool,
            tc.tile_pool(name="slices", bufs=9) as slice_pool,
            tc.tile_pool(name="stage", bufs=3) as stage_pool,
            tc.tile_pool(name="tmp", bufs=4) as tmp_pool,
            tc.tile_pool(name="psum", bufs=8, space="PSUM") as psum_pool,
        ):
            wsp_sb = wpool.tile([128, 2 * 64], BF16, name="wsp_sb", tag="wsp")
            wtp_sb = wpool.tile([128, 5 * 64], BF16, name="wtp_sb", tag="wtp")
            nc.sync.dma_start(out=wsp_sb[:], in_=wsp[:])
            nc.sync.dma_start(out=wtp_sb[:], in_=wtp[:])

            for zb in range(NZB):
                z0 = zb * ZB
                # ---- input slabs: one [72, ZB,XI,Y] tile per t ----
                slabs = []
                for t in range(T):
                    sl = slab_pool.tile([NR, ZB * XI * Y], BF16, name="sl", tag="sl")
                    sl_v = sl.rearrange("p (z x y) -> p z x y", z=ZB, x=XI, y=Y)
                    nc.sync.dma_start(out=sl_v[:, :, :, :], in_=xin[:, t, z0:z0 + ZB])
                    slabs.append((sl, sl_v))

                # ---- spatial phase ----
                slices = []
                for t in range(T):
                    slc = slice_pool.tile([128, ZB * 512], BF16, name="slc", tag="slc")
                    slices.append(slc)
                    _, sl_v = slabs[t]
                    for z in range(ZB):
                        bank = psum_pool.tile([128, 512], F32, name="ps", tag="ps")
                        for j in range(2):
                            nc.tensor.matmul(
                                out=bank[64 * j:64 * j + 64, :],
                                lhsT=wsp_sb[0:NR, 0:64],
                                rhs=sl_v[0:NR, z, 0:XC, 32 * j:32 * j + 32],
                                start=True,
                                stop=False,
                                tile_position=(0, 64 * j),
                            )
                        for j in range(2):
                            nc.tensor.matmul(
                                out=bank[64 * j:64 * j + 64, :],
                                lhsT=wsp_sb[0:36, 64:128],
                                rhs=sl_v[0:36, z, 2:2 + XC, 32 * j:32 * j + 32],
                                start=False,
                                stop=True,
                                tile_position=(0, 64 * j),
                            )
                        nc.scalar.copy(
                            slices[t][:, z * 512:(z + 1) * 512], bank[:, :]
                        )

                # ---- temporal phase ----
                for t in range(T):
                    stg = stage_pool.tile([128, ZB * 512], BF16, name="stg", tag="stg")
                    taps = _temporal_taps(t)
                    for zp in range(ZB // 2):
                        ze, zo = 2 * zp, 2 * zp + 1
                        bkA = psum_pool.tile([128, 512], F32, name="ps", tag="ps")
                        bkB = psum_pool.tile([128, 512], F32, name="ps", tag="ps")
                        for a, (s, v) in enumerate(taps):
                            st = a == 0
                            sp = a == len(taps) - 1
                            vsl = slices[s]
                            c0, c1 = v * 64, (v + 1) * 64
                            nc.tensor.matmul(
                                out=bkA[0:64, :],
                                lhsT=wtp_sb[0:64, c0:c1],
                                rhs=vsl[0:64, ze * 512:(ze + 1) * 512],
                                start=st, stop=sp, tile_position=(0, 0),
                            )
                            nc.tensor.matmul(
                                out=bkA[64:128, :],
                                lhsT=wtp_sb[64:128, c0:c1],
                                rhs=vsl[64:128, ze * 512:(ze + 1) * 512],
                                start=st, stop=sp, tile_position=(64, 64),
                            )
                            nc.tensor.matmul(
                                out=bkB[64:128, :],
                                lhsT=wtp_sb[0:64, c0:c1],
                                rhs=vsl[0:64, zo * 512:(zo + 1) * 512],
                                start=st, stop=sp, tile_position=(0, 64),
                            )
                            nc.tensor.matmul(
                                out=bkB[0:64, :],
                                lhsT=wtp_sb[64:128, c0:c1],
                                rhs=vsl[64:128, zo * 512:(zo + 1) * 512],
                                start=st, stop=sp, tile_position=(64, 0),
                            )
                        # evacuate: cast to bf16, then 32x32 block transposes;
                        # bkB halves are swapped (j1 in partitions 0-63) by
                        # the col tiling
                        tmpA = tmp_pool.tile([128, 512], BF16, name="tmpA", tag="tmpA")
                        tmpB = tmp_pool.tile([128, 512], BF16, name="tmpB", tag="tmpB")
                        nc.scalar.copy(tmpA[:, :], bkA[:, :])
                        nc.vector.tensor_copy(tmpB[:, :], bkB[:, :])
                        nc.vector.transpose(
                            stg[:, ze * 512:(ze + 1) * 512], tmpA[:, :]
                        )
                        nc.vector.transpose(
                            stg[0:64, zo * 512:(zo + 1) * 512], tmpB[64:128, :]
                        )
                        nc.vector.transpose(
                            stg[64:128, zo * 512:(zo + 1) * 512], tmpB[0:64, :]
                        )
                    # stage layout: partition 32*(2j+q') + y', free z*512+x*32+f
                    for ab in range(4):
                        dst_t = out_r if ab % 2 == 0 else out_i
                        u = ab // 2
                        dst = dst_t[t, z0:z0 + ZB, 32 * u:32 * u + 32].rearrange(
                            "z r x f -> r z x f"
                        )
                        src = stg[32 * ab:32 * ab + 32, :].rearrange(
                            "p (z x f) -> p z x f", z=ZB, x=XC, f=F
                        )
                        nc.sync.dma_start(out=dst, in_=src)

    nc.finalize()
    return nc


def _prep_inputs(xr, xi, wxyz_r, wxyz_i, wt_r, wt_i):
    xr = np.asarray(xr, np.float32)
    xi = np.asarray(xi, np.float32)

    wsr, wsi = _project(np.asarray(wxyz_r, np.float64), np.asarray(wxyz_i, np.float64), True)
    wtr, wti = _project(np.asarray(wt_r, np.float64), np.asarray(wt_i, np.float64), False)
    wsp = _spatial_lhsT(wsr, wsi)
    wtp = _temporal_lhsT(wtr, wti)

    pads = [(0, 0), (0, 0), (1, 1), (1, 1), (1, 1), (0, 0)]
    xp = np.stack([np.pad(xr, pads, mode="symmetric"),
                   np.pad(xi, pads, mode="symmetric")])  # [ri2, B, T, ZP, YP, XP, C]
    xp = xp.astype(BF16NP)
    gsel = np.minimum(np.arange(XI) + 1, XI - 1)
    in_maps = []
    for core in range(8):
        b, cx = divmod(core, NXC)
        xs = xp[:, b, :, :, :, XC * cx:XC * cx + XI, :]   # [ri2, T, ZP, YP, XI, C]
        xin = np.empty((NR, T, Z, XI, Y), BF16NP)
        for dz in range(KZ):
            for dy in range(KY):
                blk = xs[:, :, dz:dz + Z, dy:dy + Y, :, :]     # [ri,T,Z,Y,XI,C]
                blk = blk.transpose(5, 0, 1, 2, 4, 3)          # [C,ri,T,Z,XI,Y]
                blk = blk.reshape(4, T, Z, XI, Y)
                r0 = ((dz * 3 + dy) * 4)
                xin[r0:r0 + 4] = blk
                xin[36 + r0:36 + r0 + 4] = blk[:, :, :, gsel, :]
        in_maps.append({"xin": xin, "wsp": wsp, "wtp": wtp})
    return in_maps


def kernel(xr, xi, wxyz_r, wxyz_i, wt_r, wt_i):
    if "nc" not in _NC_CACHE:
        _NC_CACHE["nc"] = build_program()
    nc = _NC_CACHE["nc"]

    in_maps = _prep_inputs(xr, xi, wxyz_r, wxyz_i, wt_r, wt_i)
    res = run_bass_kernel_spmd(nc, in_maps, list(range(8)))

    yr = np.empty((B, T, Z, Y, X, F), np.float32)
    yi = np.empty((B, T, Z, Y, X, F), np.float32)
    for core in range(8):
        b, cx = divmod(core, NXC)
        yr[b, :, :, :, XC * cx:XC * cx + XC, :] = res.results[core]["out_r"].astype(np.float32)
        yi[b, :, :, :, XC * cx:XC * cx + XC, :] = res.results[core]["out_i"].astype(np.float32)
    return yr, yi


# revision 8
# speedup vs baseline: 1.2326x; 1.0002x over previous
"""Complex 3D+temporal conv (ComplexPadConv3Dt) on 8 Trainium2 NeuronCores.

Strategy (hardcoded for B=2, T=8, Z=20, Y=64, X=64, C=2, F1=F=32, k=3):
 - Pure data-parallel sharding: 8 cores = B(2) x X-quarters(4). Each core
   computes its (b, 16-wide x slab) including halo; no collectives.
 - All matmuls bf16 (rel err ~5e-3 vs the 2e-2 gate), PSUM accumulates f32.
 - The PE array is output-drain-bound (~128 PSUM elems/cycle shared across
   concurrent tiles), so the design minimizes matmuls per output tile:
   * Spatial conv: 2 accumulating matmuls per [64,512] output tile:
     K=72 covering (dz,dy) x (c,ri) taps for dx in {0,1} (dx=1 rows are
     x-preshifted copies), then K=36 covering dx=2 via a free-dim x offset
     on the same 36 base rows. dz/dy shifts are baked into the DRAM relayout.
   * The two y-halves (j) of each (t,z) run as concurrent column tiles
     (cols 0-63 / 64-127) accumulating into one [128,512] PSUM bank.
   * Temporal conv: K=64 contraction (q,f1), 3 taps accumulated; per tap a
     4-matmul wave covers (2 z) x (2 j) on disjoint PE quadrants.
 - Spatial PSUM evacuated by ScalarE as bf16 slices; temporal PSUM
   evacuated via DVE 32x32 block transposes giving per-partition
   (x,f)-contiguous 1KB HBM runs; outputs stored bf16, upcast on host.
"""

import numpy as np
import ml_dtypes

import concourse.bass as bass
import concourse.bacc as bacc
import concourse.mybir as mybir
from concourse import tile
from concourse.bass_utils import run_bass_kernel_spmd

# Problem constants
B, T, Z, Y, X, C = 2, 8, 20, 64, 64, 2
F1, F = 32, 32
KZ = KY = KX = 3
KT = 3

# Sharding / tiling
XC = 16          # output x columns per core
NXC = X // XC    # 4 x-chunks
XI = XC + 2      # input x columns per core (halo)
ZB = 4           # z rows per block
NZB = Z // ZB    # 5 blocks
NR = 72          # spatial contraction rows: 36 shiftable + 36 dx=1-preshifted

F32 = mybir.dt.float32
BF16 = mybir.dt.bfloat16
BF16NP = ml_dtypes.bfloat16

_NC_CACHE = {}


def _project(wr, wi, zero_mean):
    wr = wr.astype(np.float64)
    wi = wi.astype(np.float64)
    ax = (0, 1, 2, 3)
    if zero_mean:
        wr = wr - wr.mean(ax, keepdims=True)
        wi = wi - wi.mean(ax, keepdims=True)
    norm = np.sqrt((wr * wr + wi * wi).sum(ax, keepdims=True))
    s = 1.0 / np.maximum(norm, 1.0)
    return wr * s, wi * s


def _spatial_lhsT(wsr, wsi):
    """[128, 2*64] f32->bf16.

    Block 0 (cols 0-63, K=72): rows r = (dz*3+dy)*4 + c*2 + ri for dx=0
    (r<36) and the same +36 for dx=1. Block 1 (cols 64-127, K=36):
    rows 0-35 for dx=2. Cols: q'*32 + f (q'=0 -> yr, q'=1 -> yi).
    """
    w = np.zeros((128, 2 * 64), np.float64)
    for blk, dxs in ((0, (0, 1)), (1, (2,))):
        for dxi, dx in enumerate(dxs):
            for dz in range(KZ):
                for dy in range(KY):
                    for c in range(C):
                        r0 = dxi * 36 + (dz * 3 + dy) * 4 + c * 2
                        col = blk * 64
                        wr = wsr[dz, dy, dx, c, :]
                        wi = wsi[dz, dy, dx, c, :]
                        w[r0 + 0, col + 0:col + 32] = wr
                        w[r0 + 0, col + 32:col + 64] = wi
                        w[r0 + 1, col + 0:col + 32] = -wi
                        w[r0 + 1, col + 32:col + 64] = wr
    return w.astype(BF16NP)


def _temporal_lhsT(wtr, wti):
    """[128, 5*64] bf16. rows 64d + q*32 + f1 (q=0 spr, 1 spi); cols q'*32 + f.

    variants v: [wt0, wt1, wt2, wt0+wt1, wt1+wt2]
    """
    wtr = wtr.reshape(KT, F1, F)
    wti = wti.reshape(KT, F1, F)
    variants = [
        (wtr[0], wti[0]),
        (wtr[1], wti[1]),
        (wtr[2], wti[2]),
        (wtr[0] + wtr[1], wti[0] + wti[1]),
        (wtr[1] + wtr[2], wti[1] + wti[2]),
    ]
    w = np.zeros((64, 5 * 64), np.float64)
    for v, (vr, vi) in enumerate(variants):
        w[0:32, v * 64 + 0:v * 64 + 32] = vr          # spr -> yr
        w[0:32, v * 64 + 32:v * 64 + 64] = vi         # spr -> yi
        w[32:64, v * 64 + 0:v * 64 + 32] = -vi        # spi -> yr
        w[32:64, v * 64 + 32:v * 64 + 64] = vr        # spi -> yi
    out = np.zeros((128, 5 * 64), np.float64)
    out[0:64] = w
    out[64:128] = w
    return out.astype(BF16NP)


def _temporal_taps(t):
    if t == 0:
        return [(0, 3), (1, 2)]
    if t == T - 1:
        return [(T - 2, 0), (T - 1, 4)]
    return [(t - 1, 0), (t, 1), (t + 1, 2)]


def build_program():
    nc = bacc.Bacc(None, target_bir_lowering=False)

    xin = nc.declare_dram_parameter("xin", [NR, T, Z, 2, XI, 32], BF16, isOutput=False)
    wsp = nc.declare_dram_parameter("wsp", [128, 2 * 64], BF16, isOutput=False)
    wtp = nc.declare_dram_parameter("wtp", [128, 5 * 64], BF16, isOutput=False)
    out_r = nc.declare_dram_parameter("out_r", [T, Z, Y, XC, F], BF16, isOutput=True)
    out_i = nc.declare_dram_parameter("out_i", [T, Z, Y, XC, F], BF16, isOutput=True)

    with tile.TileContext(nc) as tc:
        with (
            tc.tile_pool(name="wpool", bufs=1) as wpool,
            tc.tile_pool(name="slabs", bufs=12) as slab_pool,
            tc.tile_pool(name="slices", bufs=9) as slice_pool,
            tc.tile_pool(name="stage", bufs=3) as stage_pool,
            tc.tile_pool(name="tmp", bufs=4) as tmp_pool,
            tc.tile_pool(name="psum", bufs=8, space="PSUM") as psum_pool,
        ):
            wsp_sb = wpool.tile([128, 2 * 64], BF16, name="wsp_sb", tag="wsp")
            wtp_sb = wpool.tile([128, 5 * 64], BF16, name="wtp_sb", tag="wtp")
            nc.sync.dma_start(out=wsp_sb[:], in_=wsp[:])
            nc.sync.dma_start(out=wtp_sb[:], in_=wtp[:])

            for zb in range(NZB):
                z0 = zb * ZB
                # ---- input slabs: one [72, ZB,XI,Y] tile per t ----
                slabs = []
                for t in range(T):
                    sl = slab_pool.tile([NR, ZB * 2 * XI * 32], BF16, name="sl", tag="sl")
                    sl_v = sl.rearrange(
                        "p (z j x y) -> p z j x y", z=ZB, j=2, x=XI, y=32
                    )
                    nc.sync.dma_start(out=sl_v[:, :, :, :, :], in_=xin[:, t, z0:z0 + ZB])
                    slabs.append((sl, sl_v))

                # ---- spatial phase ----
                slices = []
                for t in range(T):
                    slc = slice_pool.tile([128, ZB * 512], BF16, name="slc", tag="slc")
                    slices.append(slc)
                    _, sl_v = slabs[t]
                    for z in range(ZB):
                        bank = psum_pool.tile([128, 512], F32, name="ps", tag="ps")
                        for j in range(2):
                            nc.tensor.matmul(
                                out=bank[64 * j:64 * j + 64, :],
                                lhsT=wsp_sb[0:NR, 0:64],
                                rhs=sl_v[0:NR, z, j, 0:XC, :],
                                start=True,
                                stop=False,
                                tile_position=(0, 64 * j),
                            )
                        for j in range(2):
                            nc.tensor.matmul(
                                out=bank[64 * j:64 * j + 64, :],
                                lhsT=wsp_sb[0:36, 64:128],
                                rhs=sl_v[0:36, z, j, 2:2 + XC, :],
                                start=False,
                                stop=True,
                                tile_position=(0, 64 * j),
                            )
                        nc.scalar.copy(
                            slices[t][:, z * 512:(z + 1) * 512], bank[:, :]
                        )

                # ---- temporal phase ----
                for t in range(T):
                    stg = stage_pool.tile([128, ZB * 512], BF16, name="stg", tag="stg")
                    taps = _temporal_taps(t)
                    for zp in range(ZB // 2):
                        ze, zo = 2 * zp, 2 * zp + 1
                        bkA = psum_pool.tile([128, 512], F32, name="ps", tag="ps")
                        bkB = psum_pool.tile([128, 512], F32, name="ps", tag="ps")
                        for a, (s, v) in enumerate(taps):
                            st = a == 0
                            sp = a == len(taps) - 1
                            vsl = slices[s]
                            c0, c1 = v * 64, (v + 1) * 64
                            nc.tensor.matmul(
                                out=bkA[0:64, :],
                                lhsT=wtp_sb[0:64, c0:c1],
                                rhs=vsl[0:64, ze * 512:(ze + 1) * 512],
                                start=st, stop=sp, tile_position=(0, 0),
                            )
                            nc.tensor.matmul(
                                out=bkA[64:128, :],
                                lhsT=wtp_sb[64:128, c0:c1],
                                rhs=vsl[64:128, ze * 512:(ze + 1) * 512],
                                start=st, stop=sp, tile_position=(64, 64),
                            )
                            nc.tensor.matmul(
                                out=bkB[64:128, :],
                                lhsT=wtp_sb[0:64, c0:c1],
                                rhs=vsl[0:64, zo * 512:(zo + 1) * 512],
                                start=st, stop=sp, tile_position=(0, 64),
                            )
                            nc.tensor.matmul(
                                out=bkB[0:64, :],
                                lhsT=wtp_sb[64:128, c0:c1],
                                rhs=vsl[64:128, zo * 512:(zo + 1) * 512],
                                start=st, stop=sp, tile_position=(64, 0),
                            )
                        # evacuate: cast to bf16, then 32x32 block transposes;
                        # bkB halves are swapped (j1 in partitions 0-63) by
                        # the col tiling
                        tmpA = tmp_pool.tile([128, 512], BF16, name="tmpA", tag="tmpA")
                        tmpB = tmp_pool.tile([128, 512], BF16, name="tmpB", tag="tmpB")
                        nc.scalar.copy(tmpA[:, :], bkA[:, :])
                        nc.vector.tensor_copy(tmpB[:, :], bkB[:, :])
                        nc.vector.transpose(
                            stg[:, ze * 512:(ze + 1) * 512], tmpA[:, :]
                        )
                        nc.vector.transpose(
                            stg[0:64, zo * 512:(zo + 1) * 512], tmpB[64:128, :]
                        )
                        nc.vector.transpose(
                            stg[64:128, zo * 512:(zo + 1) * 512], tmpB[0:64, :]
                        )
                    # stage layout: partition 32*(2j+q') + y', free z*512+x*32+f
                    for ab in range(4):
                        dst_t = out_r if ab % 2 == 0 else out_i
                        u = ab // 2
                        dst = dst_t[t, z0:z0 + ZB, 32 * u:32 * u + 32].rearrange(
                            "z r x f -> r z x f"
                        )
                        src = stg[32 * ab:32 * ab + 32, :].rearrange(
                            "p (z x f) -> p z x f", z=ZB, x=XC, f=F
                        )
                        nc.sync.dma_start(out=dst, in_=src)

    nc.finalize()
    return nc


def _prep_inputs(xr, xi, wxyz_r, wxyz_i, wt_r, wt_i):
    xr = np.asarray(xr, np.float32)
    xi = np.asarray(xi, np.float32)

    wsr, wsi = _project(np.asarray(wxyz_r, np.float64), np.asarray(wxyz_i, np.float64), True)
    wtr, wti = _project(np.asarray(wt_r, np.float64), np.asarray(wt_i, np.float64), False)
    wsp = _spatial_lhsT(wsr, wsi)
    wtp = _temporal_lhsT(wtr, wti)

    pads = [(0, 0), (0, 0), (1, 1), (1, 1), (1, 1), (0, 0)]
    xp = np.stack([np.pad(xr, pads, mode="symmetric"),
                   np.pad(xi, pads, mode="symmetric")])  # [ri2, B, T, ZP, YP, XP, C]
    xp = xp.astype(BF16NP)
    gsel = np.minimum(np.arange(XI) + 1, XI - 1)
    in_maps = []
    for core in range(8):
        b, cx = divmod(core, NXC)
        xs = xp[:, b, :, :, :, XC * cx:XC * cx + XI, :]   # [ri2, T, ZP, YP, XI, C]
        xin = np.empty((NR, T, Z, 2, XI, 32), BF16NP)
        for dz in range(KZ):
            for dy in range(KY):
                blk = xs[:, :, dz:dz + Z, dy:dy + Y, :, :]     # [ri,T,Z,Y,XI,C]
                blk = blk.reshape(2, T, Z, 2, 32, XI, C)       # y -> (j, y')
                blk = blk.transpose(6, 0, 1, 2, 3, 5, 4)       # [C,ri,T,Z,j,XI,y']
                blk = blk.reshape(4, T, Z, 2, XI, 32)
                r0 = ((dz * 3 + dy) * 4)
                xin[r0:r0 + 4] = blk
                xin[36 + r0:36 + r0 + 4] = blk[:, :, :, :, gsel, :]
        in_maps.append({"xin": xin, "wsp": wsp, "wtp": wtp})
    return in_maps


def kernel(xr, xi, wxyz_r, wxyz_i, wt_r, wt_i):
    if "nc" not in _NC_CACHE:
        _NC_CACHE["nc"] = build_program()
    nc = _NC_CACHE["nc"]

    in_maps = _prep_inputs(xr, xi, wxyz_r, wxyz_i, wt_r, wt_i)
    res = run_bass_kernel_spmd(nc, in_maps, list(range(8)))

    yr = np.empty((B, T, Z, Y, X, F), np.float32)
    yi = np.empty((B, T, Z, Y, X, F), np.float32)
    for core in range(8):
        b, cx = divmod(core, NXC)
        yr[b, :, :, :, XC * cx:XC * cx + XC, :] = res.results[core]["out_r"].astype(np.float32)
        yi[b, :, :, :, XC * cx:XC * cx + XC, :] = res.results[core]["out_i"].astype(np.float32)
    return yr, yi
